# revision 6
# baseline (speedup 1.0000x reference)
"""BottleneckAttention3D kernel for 8 Trainium2 NeuronCores.

Reference computation (per batch b):
    h = GroupNorm(x)                      # [C, N], C=128, N=4096, 8 groups
    q = wq @ h + bq ; k = wk @ h + bk ; v = wv @ h + bv
    attn = softmax(q.T k / sqrt(C))       # [N, N]
    out = v attn.T                        # [C, N]
    y = x + wp @ out + bp

Sharding: 8 cores = 2 batches x 4 query blocks of NQ=1024 tokens.
Each core recomputes groupnorm + K/V for its whole batch (cheap), computes
Q only for its query block, and runs a flash-attention-style loop over the
32 key blocks of 128 tokens, keeping scores in PSUM/SBUF only.

Softmax uses no row-max subtraction (scores are O(10), safe in f32) so the
denominator is sum(exp). exp() is computed in [key, query] layout on the
scalar engine; the denominator is accumulated with vector-engine adds and
one ones-vector matmul for the final cross-partition reduction.

Matmuls use float32r (TF32-like fast fp32 path, 1 cycle/row for N>=256).
"""

import sys

sys.path.insert(0, "/opt/trn_rl_repo")

import numpy as np

B = 2
C = 128
N = 4096  # 16*16*16 tokens
NQ = N // 4  # query block per core (1024)
GROUPS = 8
EPS = 1e-5
CH = 512  # free-dim chunk for x/h/k tiles
NCH = N // CH  # 8
MB = N // 128  # 32 key blocks
SCALE = 1.0 / np.sqrt(np.float32(C))

_CACHE = {}


def _build():
    import concourse.bacc as bacc
    import concourse.mybir as mybir
    import concourse.tile as tile

    F32 = mybir.dt.float32
    F32R = mybir.dt.float32r
    Exp = mybir.ActivationFunctionType.Exp
    Sqrt = mybir.ActivationFunctionType.Sqrt
    mult = mybir.AluOpType.mult
    add = mybir.AluOpType.add

    nc = bacc.Bacc("TRN2", target_bir_lowering=False, debug=False)

    # ---- DRAM I/O ----
    xb_d = nc.dram_tensor("xb", [C, N], F32, kind="ExternalInput")
    xs_d = nc.dram_tensor("xs", [C, NQ], F32, kind="ExternalInput")
    gm_d = nc.dram_tensor("gm", [C, C], F32, kind="ExternalInput")
    wqt_d = nc.dram_tensor("wqt", [C, C], F32R, kind="ExternalInput")
    wkt_d = nc.dram_tensor("wkt", [C, C], F32R, kind="ExternalInput")
    wvt_d = nc.dram_tensor("wvt", [C, C], F32R, kind="ExternalInput")
    wpt_d = nc.dram_tensor("wpt", [C, C], F32R, kind="ExternalInput")
    bq_d = nc.dram_tensor("bq", [C, 1], F32, kind="ExternalInput")
    bk_d = nc.dram_tensor("bk", [C, 1], F32, kind="ExternalInput")
    bp_d = nc.dram_tensor("bp", [C, 1], F32, kind="ExternalInput")
    bvb_d = nc.dram_tensor("bvb", [C, 4, 128], F32, kind="ExternalInput")
    gam_d = nc.dram_tensor("gam", [C, 1], F32, kind="ExternalInput")
    bet_d = nc.dram_tensor("bet", [C, 1], F32, kind="ExternalInput")
    onc_d = nc.dram_tensor("onc", [C, 1], F32R, kind="ExternalInput")
    onr_d = nc.dram_tensor("onr", [1, C], F32R, kind="ExternalInput")
    y_d = nc.dram_tensor("y", [C, NQ], F32, kind="ExternalOutput")

    with tile.TileContext(nc) as tc:
        with (
            tc.tile_pool(name="cst", bufs=1) as cst,
            tc.tile_pool(name="xp", bufs=1) as xp,
            tc.tile_pool(name="ep", bufs=3) as ep,
            tc.tile_pool(name="ps2", bufs=2, space="PSUM") as ps2,
            tc.tile_pool(name="psm", bufs=2, space="PSUM") as psm,
            tc.tile_pool(name="pso", bufs=1, space="PSUM") as pso,
        ):
            # ---- constant loads ----
            def ld(name, dram, shape, dt):
                t = cst.tile(shape, dt, tag=name)
                nc.sync.dma_start(t, dram[tuple(slice(None) for _ in shape)])
                return t

            GM = ld("gm", gm_d, [C, C], F32)
            WQT = ld("wqt", wqt_d, [C, C], F32R)
            WKT = ld("wkt", wkt_d, [C, C], F32R)
            WVT = ld("wvt", wvt_d, [C, C], F32R)
            WPT = ld("wpt", wpt_d, [C, C], F32R)
            BQ = ld("bq", bq_d, [C, 1], F32)
            BK = ld("bk", bk_d, [C, 1], F32)
            BP = ld("bp", bp_d, [C, 1], F32)
            BVB = ld("bvb", bvb_d, [C, 4, 128], F32)
            GAM = ld("gam", gam_d, [C, 1], F32)
            BET = ld("bet", bet_d, [C, 1], F32)
            ONC = ld("onc", onc_d, [C, 1], F32R)
            ONR = ld("onr", onr_d, [1, C], F32R)
            XS = ld("xs", xs_d, [C, NQ], F32)

            # ---- load x chunks + bn stats ----
            X = []
            ST = cst.tile([C, NCH, 6], F32, tag="st")
            for j in range(NCH):
                xt = xp.tile([C, CH], F32, tag=f"x{j}")
                nc.sync.dma_start(xt, xb_d[:, j * CH : (j + 1) * CH])
                X.append(xt)
                nc.vector.bn_stats(out=ST[:, j, :], in_=xt)
            MV = cst.tile([C, 2], F32, tag="mv")
            nc.vector.bn_aggr(out=MV, in_=ST)

            # per-partition [mean, E[x^2]]
            S2 = cst.tile([C, 2], F32, tag="s2")
            T0 = cst.tile([C, 1], F32, tag="t0")
            nc.vector.tensor_copy(S2[:, 0:1], MV[:, 0:1])
            nc.vector.tensor_mul(T0, MV[:, 0:1], MV[:, 0:1])
            nc.vector.tensor_add(S2[:, 1:2], T0, MV[:, 1:2])

            # group aggregate: gstats[p] = [mean_g, E_g[x^2]] for p's group
            PG = ps2.tile([C, 2], F32, tag="ps2")
            nc.tensor.matmul(PG, GM, S2, start=True, stop=True)
            GS = cst.tile([C, 2], F32, tag="gs")
            nc.vector.tensor_copy(GS, PG)
            T1 = cst.tile([C, 1], F32, tag="t1")
            VG = cst.tile([C, 1], F32, tag="vg")
            nc.vector.tensor_mul(T1, GS[:, 0:1], GS[:, 0:1])
            nc.vector.tensor_sub(VG, GS[:, 1:2], T1)
            EPST = cst.tile([C, 1], F32, tag="epst")
            nc.vector.memset(EPST, float(EPS))
            SD = cst.tile([C, 1], F32, tag="sd")
            nc.scalar.activation(SD, VG, Sqrt, bias=EPST)
            RSTD = cst.tile([C, 1], F32, tag="rstd")
            nc.vector.reciprocal(RSTD, SD)
            SC = cst.tile([C, 1], F32, tag="sc")
            TB0 = cst.tile([C, 1], F32, tag="tb0")
            TB = cst.tile([C, 1], F32, tag="tb")
            nc.vector.tensor_mul(SC, RSTD, GAM)
            nc.vector.tensor_mul(TB0, GS[:, 0:1], SC)
            nc.vector.tensor_sub(TB, BET, TB0)

            # ---- normalized h ----
            H = []
            for j in range(NCH):
                ht = xp.tile([C, CH], F32R, tag=f"h{j}")
                nc.vector.tensor_scalar(
                    out=ht, in0=X[j], scalar1=SC, scalar2=TB, op0=mult, op1=add
                )
                H.append(ht)
            HQ = cst.tile([C, NQ], F32R, tag="hq")
            nc.vector.tensor_scalar(
                out=HQ, in0=XS, scalar1=SC, scalar2=TB, op0=mult, op1=add
            )

            # ---- K, V, Q ----
            K = []
            for j in range(NCH):
                pk = ps2.tile([C, CH], F32, tag="ps2")
                nc.tensor.matmul(pk, WKT, H[j], start=True, stop=True)
                kt = xp.tile([C, CH], F32R, tag=f"k{j}")
                nc.vector.tensor_scalar_add(kt, pk, BK)
                K.append(kt)
            V = []
            for g in range(NCH):
                pv = ps2.tile([C, 4, 128], F32, tag="ps2")
                for u in range(4):
                    nc.tensor.matmul(
                        pv[:, u, :],
                        H[g][:, u * 128 : (u + 1) * 128],
                        WVT,
                        start=True,
                        stop=True,
                    )
                vt = xp.tile([C, 4, 128], F32R, tag=f"v{g}")
                nc.vector.tensor_add(vt, pv, BVB)
                V.append(vt)
            PQ = psm.tile([C, NQ], F32, tag="ps")
            for h in range(2):
                sl = slice(h * 512, (h + 1) * 512)
                nc.tensor.matmul(PQ[:, sl], WQT, HQ[:, sl], start=True, stop=True)
            QT = cst.tile([C, NQ], F32R, tag="qt")
            nc.vector.tensor_scalar_add(QT, PQ, BQ)

            # ---- main attention loop over 32 key blocks ----
            PO = pso.tile([C, NQ], F32, tag="po")
            ACC = [
                cst.tile([C, NQ], F32R, tag="acc0", name="acc0"),
                cst.tile([C, NQ], F32R, tag="acc1", name="acc1"),
            ]
            for i in range(MB):
                g, u = i // 4, i % 4
                kblk = K[g][:, u * 128 : (u + 1) * 128]
                psS = psm.tile([C, NQ], F32, tag="ps")
                for h in range(2):
                    sl = slice(h * 512, (h + 1) * 512)
                    nc.tensor.matmul(psS[:, sl], kblk, QT[:, sl], start=True, stop=True)
                E = ep.tile([C, NQ], F32R, tag="e")
                nc.scalar.activation(E, psS, Exp)
                for h in range(2):
                    sl = slice(h * 512, (h + 1) * 512)
                    nc.tensor.matmul(
                        PO[:, sl],
                        V[g][:, u, :],
                        E[:, sl],
                        start=(i == 0),
                        stop=(i == MB - 1),
                    )
                if i < 2:
                    nc.vector.tensor_copy(ACC[i], E)
                else:
                    nc.vector.tensor_add(ACC[i % 2], ACC[i % 2], E)

            # ---- softmax denominator + normalization + projection ----
            ACCF = cst.tile([C, NQ], F32R, tag="accf")
            nc.vector.tensor_add(ACCF, ACC[0], ACC[1])
            PD = psm.tile([1, NQ], F32, tag="ps")
            for h in range(2):
                sl = slice(h * 512, (h + 1) * 512)
                nc.tensor.matmul(PD[:, sl], ONC, ACCF[:, sl], start=True, stop=True)
            REC = cst.tile([1, NQ], F32R, tag="rec")
            with nc.allow_low_precision(reason="f32r rounding of 1/denom is fine"):
                nc.vector.reciprocal(REC, PD)
            PB = psm.tile([C, NQ], F32, tag="ps")
            for h in range(2):
                sl = slice(h * 512, (h + 1) * 512)
                nc.tensor.matmul(PB[:, sl], ONR, REC[:, sl], start=True, stop=True)
            RB = cst.tile([C, NQ], F32, tag="rb")
            nc.vector.tensor_copy(RB, PB)
            OUTN = cst.tile([C, NQ], F32R, tag="outn")
            nc.vector.tensor_mul(OUTN, PO, RB)
            PP = psm.tile([C, NQ], F32, tag="ps")
            for h in range(2):
                sl = slice(h * 512, (h + 1) * 512)
                nc.tensor.matmul(PP[:, sl], WPT, OUTN[:, sl], start=True, stop=True)
            XSB = cst.tile([C, NQ], F32, tag="xsb")
            nc.vector.tensor_scalar_add(XSB, XS, BP)
            Y = cst.tile([C, NQ], F32, tag="y")
            nc.vector.tensor_add(Y, PP, XSB)
            nc.sync.dma_start(y_d[:, :], Y)

    nc.compile()
    return nc


def _get_nc():
    if "nc" not in _CACHE:
        _CACHE["nc"] = _build()
    return _CACHE["nc"]


def kernel(
    x,
    gamma,
    beta,
    wq,
    bq,
    wk,
    bk,
    wv,
    bv,
    wp,
    bp,
    _results_hook=None,
    _run_kwargs=None,
    **_unused,
):
    from concourse.bass_utils import run_bass_kernel_spmd

    f = np.float32
    x = np.ascontiguousarray(np.asarray(x, dtype=f))
    Bx, Cx, D, Hh, W = x.shape
    xr = x.reshape(Bx, Cx, D * Hh * W)

    gamma = np.asarray(gamma, f).reshape(C, 1)
    beta = np.asarray(beta, f).reshape(C, 1)
    wq = np.asarray(wq, f)
    wk = np.asarray(wk, f)
    wv = np.asarray(wv, f)
    wp = np.asarray(wp, f)
    bq = np.asarray(bq, f).reshape(C, 1)
    bk = np.asarray(bk, f).reshape(C, 1)
    bv = np.asarray(bv, f)
    bp = np.asarray(bp, f).reshape(C, 1)

    scale = f(1.0) / np.sqrt(f(C))
    wqt = np.ascontiguousarray(wq.T * scale)
    bqs = np.ascontiguousarray(bq * scale)
    wkt = np.ascontiguousarray(wk.T)
    wvt = np.ascontiguousarray(wv.T)
    wpt = np.ascontiguousarray(wp.T)
    bvb = np.ascontiguousarray(np.broadcast_to(bv[None, None, :], (C, 4, 128)), f)

    # group-average matrix: 1/16 within each 16-channel group
    gsz = C // GROUPS
    gm = np.kron(np.eye(GROUPS, dtype=f), np.full((gsz, gsz), 1.0 / gsz, f))

    shared = {
        "gm": gm,
        "wqt": wqt,
        "wkt": wkt,
        "wvt": wvt,
        "wpt": wpt,
        "bq": bqs,
        "bk": bk,
        "bp": bp,
        "bvb": bvb,
        "gam": gamma,
        "bet": beta,
        "onc": np.ones((C, 1), f),
        "onr": np.ones((1, C), f),
    }
    in_maps = []
    for core in range(8):
        b, s = core // 4, core % 4
        in_maps.append(
            {
                "xb": np.ascontiguousarray(xr[b]),
                "xs": np.ascontiguousarray(xr[b][:, s * NQ : (s + 1) * NQ]),
                **shared,
            }
        )

    nc = _get_nc()
    res = run_bass_kernel_spmd(
        nc, in_maps, core_ids=list(range(8)), **(_run_kwargs or {})
    )
    if _results_hook is not None:
        _results_hook(res)

    out = np.empty((Bx, Cx, D * Hh * W), f)
    for core in range(8):
        b, s = core // 4, core % 4
        out[b][:, s * NQ : (s + 1) * NQ] = res.results[core]["y"]
    return out.reshape(Bx, Cx, D, Hh, W)
